# revision 1
# baseline (speedup 1.0000x reference)
"""3-layer GAT + linear head on 8 TRN2 NeuronCores (Bass/Tile), bf16 edition.

Sharding (follows the problem hint):
  - Nodes split into 8 contiguous blocks of 6250; core k owns block k and
    every edge whose destination lies in its block.
  - Per layer: each core computes H = X @ W.T (+ attention projections) for
    its own nodes, AllGathers the rows into a replicated bf16 node table,
    then processes its edges per 128-destination tile:
      * dma_gather of bf16 source rows (round-robin over 4 SWDGE queues),
      * per-edge attention w = exp(leakyrelu(a_s[src] + a_d[dst])) written
        into the row's a_s slot,
      * segment softmax + weighted sum via one-hot matmuls on TensorE with
        bf16 stationaries (S01[e,d] = (dst_loc[e]==d)),
      * epilogue: normalize, bias, ELU, PE-transpose, and the NEXT layer's
        X @ W.T fused in (h1T/h2T/h3T never round-trip DRAM).
  - Layer-1 h uses an interleaved (channel, head) column order so the
    per-edge/-dst broadcasts are unit-stride on the vector engine; weight
    matrices are permuted host-side to compensate.
  - a_s.h per node rides in the gathered row (free for layers 2/3 in the
    256B-min gather row); a_d[dst] is broadcast edge-wise via a tiny matmul
    against the PE-transposed one-hot.

Self-contained; hardcodes shapes for N=50000, E=800000, D_IN=128, HID=64,
HEADS=8, D_OUT=10.
"""
import os
import numpy as np
import ml_dtypes

import concourse.bass as bass
import concourse.mybir as mybir
import concourse.tile as tile
from concourse import bacc
from concourse.bass_utils import run_bass_kernel_spmd
from concourse.masks import make_identity

N = 50000
E = 800000
NCORES = 8
VP = N // NCORES          # 6250 nodes per core
P = 128
NT = (VP + P - 1) // P    # 49 dst tiles per core (last has 106 rows)
NTP = NT * P              # 6272
HALF = N // 2             # 25000
D_IN = 128
HID = 64
HEADS = 8
D_OUT = 10
R1 = 640                  # layer-1 row: h(512 interleaved) | w/a_s(8) | pad
R2 = 128                  # layer-2/3 row: h(64) | a_s(1) | pad  (256B min)
NQ = 4                    # swdge queues for gather descriptor generation

f32 = mybir.dt.float32
bf16 = mybir.dt.bfloat16
i16 = mybir.dt.int16
AT = mybir.AluOpType
AF = mybir.ActivationFunctionType

BF = ml_dtypes.bfloat16


def _prep_edges(edge_index):
    src = np.concatenate([np.asarray(edge_index[0]), np.arange(N)]).astype(np.int64)
    dst = np.concatenate([np.asarray(edge_index[1]), np.arange(N)]).astype(np.int64)

    per_core = []
    maxch = [0, 0]
    for k in range(NCORES):
        m = (dst >= k * VP) & (dst < (k + 1) * VP)
        s_k = src[m]
        dloc = dst[m] - k * VP
        t_k = dloc // P
        w_k = dloc % P
        tiles = []
        for t in range(NT):
            sel = t_k == t
            ss, ww = s_k[sel], w_k[sel]
            groups = []
            for g in range(2):
                gm = (ss < HALF) if g == 0 else (ss >= HALF)
                li = (ss[gm] - g * HALF).astype(np.int64)
                groups.append((li, ww[gm].astype(np.int64)))
                maxch[g] = max(maxch[g], (len(li) + P - 1) // P)
            tiles.append(groups)
        per_core.append(tiles)

    chs = [max(c, 1) for c in maxch]
    idx_arrs, dst_arrs = [], []
    for g in range(2):
        ch = chs[g]
        ia, da = [], []
        for k in range(NCORES):
            A = np.zeros((NT, P, ch * 8), np.int16)
            D = np.full((NT, P, ch), -1.0, np.float32)
            for t in range(NT):
                li, ww = per_core[k][t][g]
                n = len(li)
                ii = np.arange(n)
                wrap = np.zeros((16, ch * 8), np.int16)
                wrap[ii % 16, ii // 16] = li.astype(np.int16)
                A[t] = np.tile(wrap, (8, 1))
                D[t, ii % P, ii // P] = ww
            ia.append(A)
            da.append(D)
        idx_arrs.append(ia)
        dst_arrs.append(da)
    return chs, idx_arrs, dst_arrs


def _edge_phase(nc, tc, layer, CHs, idx_ins, dst_ins, hfull, Rrow, heads,
                ad_sb, iotab, identb, brep, rows_of, nxt):
    """Edge aggregation for one GAT layer + fused next-layer matmul.

    nxt: (W_next_ap_fn, ncb, ad_next, hloc_next) for layers 1/2 where
         W_next_ap_fn(cb) yields the [128, 66] rhs block; for layer 3:
         ("out", Wcb, bcr, out_d).
    """
    HC = 512 if layer == 1 else HID
    CHa, CHb = CHs
    CHM = max(CHa, CHb)
    with tc.tile_pool(name=f"e{layer}", bufs=8) as ep, \
         tc.tile_pool(name=f"e{layer}o", bufs=2) as op, \
         tc.tile_pool(name=f"e{layer}w", bufs=8) as wp, \
         tc.tile_pool(name=f"e{layer}s", bufs=CHM + 2) as sp, \
         tc.tile_pool(name=f"e{layer}p1", bufs=2, space="PSUM") as pp, \
         tc.tile_pool(name=f"e{layer}p2", bufs=2, space="PSUM") as pa, \
         tc.tile_pool(name=f"e{layer}p3", bufs=1, space="PSUM") as po:
        PF = 3  # idx/dst prefetch distance (tiles)
        pend = {}

        def load_tile(tt):
            for g in (0, 1):
                CH = CHs[g]
                idxt = wp.tile([P, CHM * 8], i16, tag="idx")
                nc.sync.dma_start(out=idxt[:, 0:CH * 8], in_=idx_ins[g][tt])
                dstt = wp.tile([P, CHM], f32, tag="dst")
                nc.sync.dma_start(out=dstt[:, 0:CH], in_=dst_ins[g][tt])
                pend[(tt, g)] = (idxt, dstt)

        for tt in range(min(PF, NT)):
            load_tile(tt)
        for t in range(NT):
            if t + PF < NT:
                load_tile(t + PF)
            if heads == 8:
                outu = po.tile([P, HC], f32, space="PSUM", tag="outu")
                ssum = po.tile([P, 8], f32, space="PSUM", tag="ssum")
            else:
                outu = po.tile([P, HID + 1], f32, space="PSUM", tag="outu")
            adT = ad_sb[:, t * heads:(t + 1) * heads]
            for g in range(2):
                CH = CHs[g]
                NIDX = CH * P
                idxt, dstt = pend.pop((t, g))
                G = ep.tile([P, CHM, Rrow], bf16, tag="G")
                half = hfull[g * HALF:(g + 1) * HALF, :]
                nc.gpsimd.dma_gather(G[:, 0:CH, :], half, idxt[:, 0:CH * 8],
                                     NIDX, NIDX, Rrow, single_packet=False,
                                     queue_num=(2 * t + g) % NQ)
                s01s = []
                estt = wp.tile([P, CHM * heads], f32, tag="estt")
                for ch in range(CH):
                    s01 = sp.tile([P, P], bf16, tag="s01")
                    nc.vector.tensor_scalar(
                        out=s01[:], in0=iotab[:], scalar1=dstt[:, ch:ch + 1],
                        scalar2=None, op0=AT.is_equal)
                    s01t_ps = pp.tile([P, P], bf16, space="PSUM", tag="s01t")
                    nc.tensor.transpose(out=s01t_ps[:], in_=s01[:],
                                        identity=identb[:])
                    s01t = wp.tile([P, P], bf16, tag="s01t_sb")
                    nc.scalar.activation(s01t[:], s01t_ps[:], AF.Copy)
                    adg_ps = pa.tile([P, heads], f32, space="PSUM", tag="adg")
                    nc.tensor.matmul(adg_ps[:], lhsT=s01t[:], rhs=adT,
                                     start=True, stop=True)
                    nc.vector.tensor_tensor(
                        out=estt[:, ch * heads:(ch + 1) * heads],
                        in0=G[:, ch, HC:HC + heads],
                        in1=adg_ps[:], op=AT.add)
                    s01s.append(s01)
                # batched leaky-relu + exp over this group's edge logits
                ef = estt[:, 0:CH * heads]
                nc.vector.scalar_tensor_tensor(
                    out=ef, in0=ef, scalar=0.2, in1=ef,
                    op0=AT.mult, op1=AT.max)
                esttb = wp.tile([P, CHM * heads], bf16, tag="esttb")
                nc.scalar.activation(esttb[:, 0:CH * heads], ef, AF.Exp)
                for ch in range(CH):
                    first = (g == 0 and ch == 0)
                    last = (g == 1 and ch == CH - 1)
                    wsl = esttb[:, ch * heads:(ch + 1) * heads]
                    if heads == 8:
                        gv = G[:, ch, 0:512].rearrange("p (c h) -> p c h", h=8)
                        wv = (wsl.to_broadcast([P, 8, 64])
                              .rearrange("p a b -> p b a"))
                        nc.vector.tensor_tensor(out=gv, in0=gv, in1=wv,
                                                op=AT.mult)
                        nc.tensor.matmul(outu[:], lhsT=s01s[ch][:],
                                         rhs=G[:, ch, 0:512],
                                         start=first, stop=last,
                                         skip_group_check=True)
                        nc.tensor.matmul(ssum[:], lhsT=s01s[ch][:],
                                         rhs=wsl,
                                         start=first, stop=last,
                                         skip_group_check=True)
                    else:
                        nc.vector.tensor_copy(G[:, ch, HID:HID + 1], wsl)
                        gv = G[:, ch, 0:HID].rearrange(
                            "p (a b) -> p a b", a=1)
                        wv = wsl.to_broadcast([P, 1, HID])
                        nc.vector.tensor_tensor(out=gv, in0=gv, in1=wv,
                                                op=AT.mult)
                        nc.tensor.matmul(outu[:], lhsT=s01s[ch][:],
                                         rhs=G[:, ch, 0:HID + 1],
                                         start=first, stop=last,
                                         skip_group_check=True)
            # ---- epilogue: normalize, bias, ELU ----
            if heads == 8:
                rec = wp.tile([P, 8], f32, tag="rec")
                nc.vector.reciprocal(rec[:], ssum[:])
                ho = op.tile([P, HC], f32, tag="ho")
                hov = ho[:].rearrange("p (c h) -> p c h", h=8)
                ouv = outu[:].rearrange("p (c h) -> p c h", h=8)
                recb = (rec[:].to_broadcast([P, 8, 64])
                        .rearrange("p a b -> p b a"))
                nc.vector.tensor_tensor(out=hov, in0=ouv, in1=recb, op=AT.mult)
            else:
                rec = wp.tile([P, 1], f32, tag="rec")
                nc.vector.reciprocal(rec[:], outu[:, HID:HID + 1])
                ho = op.tile([P, HC], f32, tag="ho")
                nc.vector.tensor_scalar(out=ho[:], in0=outu[:, 0:HID],
                                        scalar1=rec[:], scalar2=None,
                                        op0=AT.mult)
            nc.vector.tensor_tensor(out=ho[:], in0=ho[:], in1=brep[:],
                                    op=AT.add)
            el = op.tile([P, HC], f32, tag="el")
            nc.vector.tensor_scalar(out=el[:], in0=ho[:], scalar1=0.0,
                                    scalar2=None, op0=AT.min)
            nc.scalar.activation(el[:], el[:], AF.Exp)
            nc.vector.scalar_tensor_tensor(
                out=ho[:], in0=ho[:], scalar=0.0, in1=el[:],
                op0=AT.max, op1=AT.add)
            # single bf16 rounding of elu(...)-1 happens here, in one op
            hob = op.tile([P, HC], bf16, tag="hob")
            nc.scalar.activation(hob[:], ho[:], AF.Copy, bias=-1.0)
            # ---- PE transpose + fused next-layer matmul ----
            r = rows_of(t)
            if layer == 1:
                W2ap, ad2, hloc2 = nxt
                tsb = op.tile([P, 512], bf16, tag="tsb")
                for cb in range(4):
                    tp_ps = pp.tile([P, P], bf16, space="PSUM", tag="s01t")
                    nc.tensor.transpose(out=tp_ps[:],
                                        in_=hob[:, cb * P:(cb + 1) * P],
                                        identity=identb[:])
                    nc.vector.tensor_copy(tsb[:, cb * P:(cb + 1) * P],
                                          tp_ps[:])
                h2_ps = pa.tile([P, 66], f32, space="PSUM", tag="hnx")
                for cb in range(4):
                    nc.tensor.matmul(h2_ps[:], lhsT=tsb[:, cb * P:(cb + 1) * P],
                                     rhs=W2ap(cb), start=(cb == 0),
                                     stop=(cb == 3), skip_group_check=True)
                hc = wp.tile([P, R2], bf16, tag="hc")
                nc.vector.tensor_copy(hc[:, 0:65], h2_ps[:, 0:65])
                nc.vector.memset(hc[:, 65:R2], 0.0)
                nc.scalar.activation(ad2[:, t:t + 1], h2_ps[:, 65:66], AF.Copy)
                nc.sync.dma_start(out=hloc2[t * P:t * P + r, :], in_=hc[:r, :])
            elif layer == 2:
                W3ap, ad3, hloc3 = nxt
                tp_ps = pp.tile([P, P], bf16, space="PSUM", tag="s01t")
                nc.tensor.transpose(out=tp_ps[:HID, :], in_=hob[:],
                                    identity=identb[:])
                tsb = wp.tile([HID, P], bf16, tag="tsb64")
                nc.vector.tensor_copy(tsb[:], tp_ps[:HID, :])
                h3_ps = pa.tile([P, 66], f32, space="PSUM", tag="hnx")
                nc.tensor.matmul(h3_ps[:], lhsT=tsb[:], rhs=W3ap,
                                 start=True, stop=True)
                hc = wp.tile([P, R2], bf16, tag="hc")
                nc.vector.tensor_copy(hc[:, 0:65], h3_ps[:, 0:65])
                nc.vector.memset(hc[:, 65:R2], 0.0)
                nc.scalar.activation(ad3[:, t:t + 1], h3_ps[:, 65:66], AF.Copy)
                nc.sync.dma_start(out=hloc3[t * P:t * P + r, :], in_=hc[:r, :])
            else:
                Wcb, bcr, out_d = nxt
                tp_ps = pp.tile([P, P], bf16, space="PSUM", tag="s01t")
                nc.tensor.transpose(out=tp_ps[:HID, :], in_=hob[:],
                                    identity=identb[:])
                tsb = wp.tile([HID, P], bf16, tag="tsb64")
                nc.vector.tensor_copy(tsb[:], tp_ps[:HID, :])
                o_ps = pa.tile([P, D_OUT], f32, space="PSUM", tag="hnx")
                nc.tensor.matmul(o_ps[:], lhsT=tsb[:], rhs=Wcb[:],
                                 start=True, stop=True)
                ob = wp.tile([P, D_OUT], f32, tag="ob")
                nc.vector.tensor_tensor(out=ob[:], in0=o_ps[:], in1=bcr[:],
                                        op=AT.add)
                nc.sync.dma_start(out=out_d[t * P:t * P + r, :], in_=ob[:r, :])


PHASE_ORDER = ["m1", "ag1", "e1", "ag2", "e2", "ag3", "e3"]


def _build_program(CHa, CHb):
    stop = os.environ.get("GAT_STOP", "e3")
    lvl = PHASE_ORDER.index(stop) + 1
    nc = bacc.Bacc("TRN2", target_bir_lowering=False, debug=False,
                   enable_asserts=False, num_devices=NCORES,
                   num_swdge_queues=NQ)

    xT_in = nc.dram_tensor("xT", [P, NTP], bf16, kind="ExternalInput")
    idxA_in = nc.dram_tensor("idxA", [NT, P, CHa * 8], i16, kind="ExternalInput")
    idxB_in = nc.dram_tensor("idxB", [NT, P, CHb * 8], i16, kind="ExternalInput")
    dstA_in = nc.dram_tensor("dstA", [NT, P, CHa], f32, kind="ExternalInput")
    dstB_in = nc.dram_tensor("dstB", [NT, P, CHb], f32, kind="ExternalInput")
    W1Tp_in = nc.dram_tensor("W1Tp", [D_IN, 512], bf16, kind="ExternalInput")
    M1sd_in = nc.dram_tensor("M1sd", [D_IN, 16], bf16, kind="ExternalInput")
    W2a_in = nc.dram_tensor("W2a", [512, 66], bf16, kind="ExternalInput")
    W3a_in = nc.dram_tensor("W3a", [HID, 66], bf16, kind="ExternalInput")
    Wcb_in = nc.dram_tensor("Wcb", [HID, D_OUT], bf16, kind="ExternalInput")
    b1p_in = nc.dram_tensor("b1p", [P, 512], f32, kind="ExternalInput")
    b2r_in = nc.dram_tensor("b2r", [P, HID], f32, kind="ExternalInput")
    b3r_in = nc.dram_tensor("b3r", [P, HID], f32, kind="ExternalInput")
    bcr_in = nc.dram_tensor("bcr", [P, D_OUT], f32, kind="ExternalInput")

    out_d = nc.dram_tensor("out", [NTP, D_OUT], f32, kind="ExternalOutput")

    dbg = os.environ.get("GAT_DEBUG") == "1"
    hcat1_loc = nc.dram_tensor("hcat1_loc", [VP, R1], bf16, kind="Internal")
    hcat1_full = nc.dram_tensor("hcat1_full", [N, R1], bf16, kind="Internal",
                                addr_space="Shared")
    hcat2_loc = nc.dram_tensor("hcat2_loc", [VP, R2], bf16, kind="Internal")
    hcat2_full = nc.dram_tensor("hcat2_full", [N, R2], bf16, kind="Internal",
                                addr_space="Shared")
    hcat3_loc = nc.dram_tensor("hcat3_loc", [VP, R2], bf16, kind="Internal")
    hcat3_full = nc.dram_tensor("hcat3_full", [N, R2], bf16, kind="Internal",
                                addr_space="Shared")

    if dbg:
        dbg1 = nc.dram_tensor("dbg1", [VP, R1], bf16, kind="ExternalOutput")
        dbg2 = nc.dram_tensor("dbg2", [VP, R2], bf16, kind="ExternalOutput")
        dbg3 = nc.dram_tensor("dbg3", [VP, R2], bf16, kind="ExternalOutput")

    def rows_of(t):
        return P if t < NT - 1 else VP - (NT - 1) * P

    rg = [list(range(NCORES))]

    with tile.TileContext(nc) as tc:
        with tc.tile_pool(name="const", bufs=1) as cs:
            identb = cs.tile([P, P], bf16)
            make_identity(nc, identb[:])
            iota32 = cs.tile([P, P], f32)
            nc.gpsimd.iota(iota32[:], pattern=[[1, P]], base=0,
                           channel_multiplier=0,
                           allow_small_or_imprecise_dtypes=True)
            iotab = cs.tile([P, P], bf16)
            nc.vector.tensor_copy(iotab[:], iota32[:])

            def c_load(name, shape, src, dtype=bf16):
                tl = cs.tile(shape, dtype, tag=name)
                nc.sync.dma_start(out=tl[:], in_=src)
                return tl

            W1Tp = c_load("W1Tp", [D_IN, 512], W1Tp_in[:])
            M1sd = c_load("M1sd", [D_IN, 16], M1sd_in[:])
            W2a = cs.tile([P, 4 * 66], bf16)
            for cb in range(4):
                nc.sync.dma_start(out=W2a[:, cb * 66:(cb + 1) * 66],
                                  in_=W2a_in[cb * P:(cb + 1) * P, :])
            W3a = c_load("W3a", [HID, 66], W3a_in[:])
            Wcb = c_load("Wcb", [HID, D_OUT], Wcb_in[:])
            b1p = c_load("b1p", [P, 512], b1p_in[:], dtype=f32)
            b2r = c_load("b2r", [P, HID], b2r_in[:], dtype=f32)
            b3r = c_load("b3r", [P, HID], b3r_in[:], dtype=f32)
            bcr = c_load("bcr", [P, D_OUT], bcr_in[:], dtype=f32)
            ad1 = cs.tile([P, NT * 8], bf16)
            ad2 = cs.tile([P, NT], bf16)
            ad3 = cs.tile([P, NT], bf16)

            # ---- M1: h1 = x @ W1.T (interleaved cols) + attn projections ----
            if lvl >= 1:
             with tc.tile_pool(name="m1", bufs=3) as mp, \
                 tc.tile_pool(name="m1p", bufs=2, space="PSUM") as mpp:
                for t in range(NT):
                    xt = mp.tile([P, P], bf16, tag="xt")
                    nc.sync.dma_start(out=xt[:],
                                      in_=xT_in[:, t * P:(t + 1) * P])
                    h_ps = mpp.tile([P, 512], f32, space="PSUM", tag="h")
                    nc.tensor.matmul(h_ps[:], lhsT=xt[:], rhs=W1Tp[:],
                                     start=True, stop=True)
                    aa_ps = mpp.tile([P, 16], f32, space="PSUM", tag="aa")
                    nc.tensor.matmul(aa_ps[:], lhsT=xt[:], rhs=M1sd[:],
                                     start=True, stop=True)
                    hc = mp.tile([P, R1], bf16, tag="hc")
                    nc.vector.tensor_copy(hc[:, 0:512], h_ps[:])
                    nc.scalar.activation(hc[:, 512:520], aa_ps[:, 0:8],
                                         AF.Copy)
                    nc.vector.memset(hc[:, 520:R1], 0.0)
                    nc.scalar.activation(ad1[:, t * 8:(t + 1) * 8],
                                         aa_ps[:, 8:16], AF.Copy)
                    r = rows_of(t)
                    nc.sync.dma_start(out=hcat1_loc[t * P:t * P + r, :],
                                      in_=hc[:r, :])
            if dbg:
                nc.sync.dma_start(out=dbg1[:], in_=hcat1_loc[:])
            if lvl >= 2:
             nc.gpsimd.collective_compute(
                "AllGather", AT.bypass, replica_groups=rg,
                ins=[hcat1_loc[:]], outs=[hcat1_full[:]])

            if lvl >= 3:
             _edge_phase(nc, tc, 1, (CHa, CHb), (idxA_in, idxB_in),
                        (dstA_in, dstB_in), hcat1_full, R1, 8, ad1,
                        iotab, identb, b1p, rows_of,
                        (lambda cb: W2a[:, cb * 66:(cb + 1) * 66], ad2,
                         hcat2_loc))
            if dbg and lvl >= 3:
                nc.sync.dma_start(out=dbg2[:], in_=hcat2_loc[:])
            if lvl >= 4:
             nc.gpsimd.collective_compute(
                "AllGather", AT.bypass, replica_groups=rg,
                ins=[hcat2_loc[:]], outs=[hcat2_full[:]])

            if lvl >= 5:
             _edge_phase(nc, tc, 2, (CHa, CHb), (idxA_in, idxB_in),
                        (dstA_in, dstB_in), hcat2_full, R2, 1, ad2,
                        iotab, identb, b2r, rows_of,
                        (W3a[:], ad3, hcat3_loc))
            if dbg and lvl >= 5:
                nc.sync.dma_start(out=dbg3[:], in_=hcat3_loc[:])
            if lvl >= 6:
             nc.gpsimd.collective_compute(
                "AllGather", AT.bypass, replica_groups=rg,
                ins=[hcat3_loc[:]], outs=[hcat3_full[:]])

            if lvl >= 7:
             _edge_phase(nc, tc, 3, (CHa, CHb), (idxA_in, idxB_in),
                        (dstA_in, dstB_in), hcat3_full, R2, 1, ad3,
                        iotab, identb, b3r, rows_of,
                        (Wcb, bcr, out_d))

    nc.compile()
    return nc


def prepare(**inputs):
    """Host preprocessing + program build; returns (nc, in_maps)."""
    x = np.asarray(inputs["x"], np.float32)
    edge_index = np.asarray(inputs["edge_index"])
    W1 = np.asarray(inputs["W1"], np.float32)
    a1_src = np.asarray(inputs["a1_src"], np.float32)
    a1_dst = np.asarray(inputs["a1_dst"], np.float32)
    b1 = np.asarray(inputs["b1"], np.float32)
    W2 = np.asarray(inputs["W2"], np.float32)
    a2_src = np.asarray(inputs["a2_src"], np.float32)
    a2_dst = np.asarray(inputs["a2_dst"], np.float32)
    b2 = np.asarray(inputs["b2"], np.float32)
    W3 = np.asarray(inputs["W3"], np.float32)
    a3_src = np.asarray(inputs["a3_src"], np.float32)
    a3_dst = np.asarray(inputs["a3_dst"], np.float32)
    b3 = np.asarray(inputs["b3"], np.float32)
    Wc = np.asarray(inputs["Wc"], np.float32)
    bc = np.asarray(inputs["bc"], np.float32)

    (CHa, CHb), idx_arrs, dst_arrs = _prep_edges(edge_index)

    # interleave permutation: new col c*8+h <- old col h*64+c
    jn = np.arange(512)
    old_idx = (jn % 8) * 64 + jn // 8

    W1T = W1.T                                     # [128, 512]
    W1h = W1.reshape(HEADS, HID, D_IN)
    M1s = np.einsum("hci,hc->ih", W1h, a1_src)     # [128, 8]
    M1d = np.einsum("hci,hc->ih", W1h, a1_dst)
    W2T = W2.T                                     # [512, 64]
    M2s = W2.T @ a2_src[0]                         # [512]
    M2d = W2.T @ a2_dst[0]
    W3T = W3.T
    M3s = W3.T @ a3_src[0]
    M3d = W3.T @ a3_dst[0]

    common = {
        "W1Tp": np.ascontiguousarray(W1T[:, old_idx]).astype(BF),
        "M1sd": np.concatenate([M1s, M1d], 1).astype(BF),
        "W2a": np.concatenate(
            [W2T, M2s[:, None], M2d[:, None]], 1)[old_idx, :].astype(BF),
        "W3a": np.concatenate(
            [W3T, M3s[:, None], M3d[:, None]], 1).astype(BF),
        "Wcb": np.ascontiguousarray(Wc.T).astype(BF),
        "b1p": np.tile(b1[old_idx], (P, 1)).astype(np.float32),
        "b2r": np.tile(b2, (P, 1)).astype(np.float32),
        "b3r": np.tile(b3, (P, 1)).astype(np.float32),
        "bcr": np.tile(bc, (P, 1)).astype(np.float32),
    }

    in_maps = []
    for k in range(NCORES):
        xk = x[k * VP:(k + 1) * VP]
        xT = np.zeros((P, NTP), np.float32)
        xT[:, :VP] = xk.T
        m = dict(common)
        m["xT"] = xT.astype(BF)
        m["idxA"] = idx_arrs[0][k]
        m["idxB"] = idx_arrs[1][k]
        m["dstA"] = dst_arrs[0][k]
        m["dstB"] = dst_arrs[1][k]
        in_maps.append(m)

    nc = _build_program(CHa, CHb)
    return nc, in_maps


def kernel(**inputs):
    nc, in_maps = prepare(**inputs)
    r = run_bass_kernel_spmd(nc, in_maps, core_ids=list(range(NCORES)))
    out = np.concatenate([r.results[k]["out"][:VP] for k in range(NCORES)], 0)
    return out.astype(np.float32)



# revision 5
# speedup vs baseline: 1.3441x; 1.3441x over previous
"""3-layer GAT + linear head on 8 TRN2 NeuronCores (Bass/Tile), bf16 edition.

Sharding (follows the problem hint):
  - Nodes split into 8 contiguous blocks of 6250; core k owns block k and
    every edge whose destination lies in its block.
  - Per layer: each core computes H = X @ W.T (+ attention projections) for
    its own nodes, AllGathers the rows into a replicated bf16 node table,
    then processes its edges per 128-destination tile:
      * dma_gather of bf16 source rows (round-robin over 4 SWDGE queues),
      * one-hot scatter matrices s01 / s01T built with ONE batched vector op
        each per tile-group (is_equal against an iota-repeat tile resp. a
        DMA-broadcast per-edge dst-slot row),
      * a_d[dst] broadcast to edges via per-chunk matmuls (lhsT = s01T
        slice) accumulated into a single PSUM strip; batched leakyrelu+exp,
      * segment softmax + weighted sum via one-hot matmuls on TensorE; for
        layers 2/3 the edge weight w is folded into the one-hot (sW) and the
        softmax denominator rides a constant-1 column of the gathered row,
      * epilogue: normalize, bias, ELU, PE-transpose, and the NEXT layer's
        X @ W.T fused in (h1T/h2T/h3T never round-trip DRAM).
  - Gather descriptor generation (the gpsimd bottleneck) is trimmed with
    per-tile chunk counts (max over cores so the program stays SPMD), -1
    tail indices (skipped by the HW), and exact num_idxs_reg.
  - Layer-1 h uses an interleaved (channel, head) column order so the
    per-edge/-dst broadcasts are unit-stride on the vector engine; weight
    matrices are permuted host-side to compensate.

Self-contained; hardcodes shapes for N=50000, E=800000, D_IN=128, HID=64,
HEADS=8, D_OUT=10.
"""
import os
import numpy as np
import ml_dtypes

import concourse.bass as bass
import concourse.mybir as mybir
import concourse.tile as tile
from concourse import bacc
from concourse.bass_utils import run_bass_kernel_spmd
from concourse.masks import make_identity

N = 50000
E = 800000
NCORES = 8
VP = N // NCORES          # 6250 nodes per core
P = 128
NT = (VP + P - 1) // P    # 49 dst tiles per core (last has 106 rows)
NTP = NT * P              # 6272
HALF = N // 2             # 25000
D_IN = 128
HID = 64
HEADS = 8
D_OUT = 10
R1 = 640                  # layer-1 row: h(512 interleaved) | a_s(8) | pad
R2 = 128                  # layer-2/3 row: h(64) | a_s(1) | one(1) | pad
NQ = 4                    # swdge queues for gather descriptor generation

f32 = mybir.dt.float32
bf16 = mybir.dt.bfloat16
i16 = mybir.dt.int16
AT = mybir.AluOpType
AF = mybir.ActivationFunctionType

BF = ml_dtypes.bfloat16

SP23 = os.environ.get("GAT_SP", "1") == "1"   # single_packet for 256B rows
USE4D = os.environ.get("GAT_4D", "1") == "1"  # batched 4D G*w multiply in e1


def _prep_edges(edge_index):
    src = np.concatenate([np.asarray(edge_index[0]), np.arange(N)]).astype(np.int64)
    dst = np.concatenate([np.asarray(edge_index[1]), np.arange(N)]).astype(np.int64)

    # per_core[k][t][g] = (local_src_idx, dst_slot) arrays
    per_core = []
    cnts = np.zeros((NCORES, NT, 2), np.int64)
    for k in range(NCORES):
        m = (dst >= k * VP) & (dst < (k + 1) * VP)
        s_k = src[m]
        dloc = dst[m] - k * VP
        t_k = dloc // P
        w_k = dloc % P
        tiles = []
        for t in range(NT):
            sel = t_k == t
            ss, ww = s_k[sel], w_k[sel]
            groups = []
            for g in range(2):
                gm = (ss < HALF) if g == 0 else (ss >= HALF)
                li = (ss[gm] - g * HALF).astype(np.int64)
                groups.append((li, ww[gm].astype(np.int64)))
                cnts[k, t, g] = len(li)
            tiles.append(groups)
        per_core.append(tiles)

    # cnt_max[t][g] identical across cores so the compiled program is SPMD
    cnt_max = np.maximum(cnts.max(axis=0), 1)        # [NT, 2]
    ch_t = (cnt_max + P - 1) // P                    # [NT, 2]
    chm = int(ch_t.max())

    # meta[t][g] = (CH_t, cnt_max) ints
    meta = [[(int(ch_t[t, g]), int(cnt_max[t, g])) for g in range(2)]
            for t in range(NT)]

    idx_arrs = [[], []]
    dst_arrs = [[], []]
    dstT_arrs = [[], []]
    for g in range(2):
        for k in range(NCORES):
            A = np.full((NT, P, chm * 8), -1, np.int16)
            D = np.full((NT, P, chm), -1.0, np.float32)
            DT = np.full((NT, chm * P), -1.0, np.float32)
            for t in range(NT):
                li, ww = per_core[k][t][g]
                n = len(li)
                ch, cm = meta[t][g]
                nid = ch * P
                # idx stream: valid edges, then zero-pads (valid) to cnt_max,
                # then -1 (skipped by HW) to CH_t*128
                iv = np.full(nid, -1, np.int16)
                iv[:n] = li.astype(np.int16)
                iv[n:cm] = 0
                ii = np.arange(nid)
                wrap = np.zeros((16, nid // 16), np.int16)
                wrap[ii % 16, ii // 16] = iv
                A[t, :, 0:nid // 16] = np.tile(wrap, (8, 1))
                ie = np.arange(n)
                D[t, ie % P, ie // P] = ww
                DT[t, 0:n] = ww
            idx_arrs[g].append(A)
            dst_arrs[g].append(D.astype(BF))
            dstT_arrs[g].append(DT.astype(BF))

    return chm, meta, idx_arrs, dst_arrs, dstT_arrs


def _edge_phase(nc, tc, layer, chm, meta, idx_ins, dst_ins, dstT_ins, hfull,
                Rrow, heads, ad_sb, iota_col, iota_rep, identb, brep, rows_of,
                nxt):
    """Edge aggregation for one GAT layer + fused next-layer matmul.

    nxt: (W_next_ap_fn, ad_next, hloc_next) for layers 1/2 where
         W_next_ap_fn(cb) yields the [128, 66] rhs block; for layer 3:
         (Wcb, bcr, out_d).
    """
    HC = 512 if layer == 1 else HID
    sp = (Rrow * 2 == 256) and SP23
    with tc.tile_pool(name=f"e{layer}", bufs=8) as ep, \
         tc.tile_pool(name=f"e{layer}o", bufs=2) as op, \
         tc.tile_pool(name=f"e{layer}w", bufs=8) as wp, \
         tc.tile_pool(name=f"e{layer}dt", bufs=4) as dp, \
         tc.tile_pool(name=f"e{layer}s", bufs=4) as s01p, \
         tc.tile_pool(name=f"e{layer}p1", bufs=2, space="PSUM") as pp, \
         tc.tile_pool(name=f"e{layer}p2", bufs=2, space="PSUM") as pa, \
         tc.tile_pool(name=f"e{layer}p3", bufs=1 if heads == 8 else 2,
                      space="PSUM") as po:
        PF = 3  # idx/dst prefetch distance (tiles)
        pend = {}

        def load_tile(tt):
            for g in (0, 1):
                CH, _cm = meta[tt][g]
                idxt = wp.tile([P, chm * 8], i16, tag="idx")
                nc.sync.dma_start(out=idxt[:, 0:CH * 8],
                                  in_=idx_ins[g][tt, :, 0:CH * 8])
                dstt = wp.tile([P, chm], bf16, tag="dst")
                nc.sync.dma_start(out=dstt[:, 0:CH], in_=dst_ins[g][tt, :, 0:CH])
                dstTb = dp.tile([P, chm * P], bf16, tag="dstT")
                nc.sync.dma_start(
                    out=dstTb[:, 0:CH * P],
                    in_=dstT_ins[g][tt:tt + 1, 0:CH * P]
                    .to_broadcast([P, CH * P]))
                pend[(tt, g)] = (idxt, dstt, dstTb)

        for tt in range(min(PF, NT)):
            load_tile(tt)
        for t in range(NT):
            if t + PF < NT:
                load_tile(t + PF)
            if heads == 8:
                # cols 0:512 numerator, 512:520 softmax denominator (ssum)
                outu = po.tile([P, HC + 8], f32, space="PSUM", tag="outu")
            else:
                # cols 0:64 numerator, 64 = sum(w*a_s) (unused), 65 = sum(w)
                outu = po.tile([P, HID + 2], f32, space="PSUM", tag="outu")
            adT = ad_sb[:, t * heads:(t + 1) * heads]
            CHb = meta[t][1][0]
            for g in range(2):
                CH, cm = meta[t][g]
                NIDX = CH * P
                idxt, dstt, dstTb = pend.pop((t, g))
                G = ep.tile([P, chm, Rrow], bf16, tag="G")
                if 2 * t + g < 8:
                    # first pass through the 8 G buffers: zero them so
                    # skipped (-1) rows never expose NaN bit patterns
                    nc.vector.memset(G[:], 0.0)
                half = hfull[g * HALF:(g + 1) * HALF, :]
                nc.gpsimd.dma_gather(G[:, 0:CH, :], half, idxt[:, 0:CH * 8],
                                     NIDX, cm, Rrow, single_packet=sp,
                                     queue_num=(2 * t + g) % NQ)
                # --- batched one-hot builds ---
                s01 = s01p.tile([P, chm, P], bf16, tag="s01")
                nc.vector.tensor_tensor(
                    out=s01[:, 0:CH, :], in0=iota_rep[:, 0:CH, :],
                    in1=dstt[:, 0:CH, None].to_broadcast([P, CH, P]),
                    op=AT.is_equal)
                s01T = s01p.tile([P, chm * P], bf16, tag="s01T")
                nc.vector.tensor_scalar(
                    out=s01T[:, 0:CH * P], in0=dstTb[:, 0:CH * P],
                    scalar1=iota_col[:], scalar2=None, op0=AT.is_equal)
                # --- a_d[dst] -> edges via matmuls into one PSUM strip ---
                estt_ps = pa.tile([P, chm * heads], f32, space="PSUM",
                                  tag="estt")
                for ch in range(CH):
                    nc.tensor.matmul(estt_ps[:, ch * heads:(ch + 1) * heads],
                                     lhsT=s01T[:, ch * P:(ch + 1) * P],
                                     rhs=adT, start=True, stop=True,
                                     skip_group_check=True)
                # --- e = leakyrelu(a_s + a_d); w = exp(e) (batched) ---
                estt = wp.tile([P, chm, heads], f32, tag="estt_sb")
                nc.vector.tensor_tensor(
                    out=estt[:, 0:CH, :],
                    in0=G[:, 0:CH, HC:HC + heads],
                    in1=estt_ps[:].rearrange("p (c h) -> p c h",
                                             h=heads)[:, 0:CH, :],
                    op=AT.add)
                ef = estt[:, 0:CH, :]
                nc.vector.scalar_tensor_tensor(
                    out=ef, in0=ef, scalar=0.2, in1=ef,
                    op0=AT.mult, op1=AT.max)
                esttb = wp.tile([P, chm, heads], bf16, tag="esttb")
                nc.scalar.activation(esttb[:, 0:CH, :], ef, AF.Exp)
                # --- weighted scatter-sum ---
                if heads == 8:
                    if USE4D:
                        gv = G[:, 0:CH, 0:512].rearrange(
                            "p c (a h) -> p c a h", h=8)
                        wv = (esttb[:, 0:CH, None, :]
                              .to_broadcast([P, CH, 64, 8]))
                        nc.vector.tensor_tensor(out=gv, in0=gv, in1=wv,
                                                op=AT.mult)
                    else:
                        for ch in range(CH):
                            gv = G[:, ch, 0:512].rearrange(
                                "p (c h) -> p c h", h=8)
                            wv = (esttb[:, ch, :].to_broadcast([P, 8, 64])
                                  .rearrange("p a b -> p b a"))
                            nc.vector.tensor_tensor(out=gv, in0=gv, in1=wv,
                                                    op=AT.mult)
                    for ch in range(CH):
                        fc = (g == 0 and ch == 0)
                        lc = (g == 1 and ch == CHb - 1)
                        nc.tensor.matmul(outu[:, 0:512], lhsT=s01[:, ch, :],
                                         rhs=G[:, ch, 0:512],
                                         start=fc, stop=lc,
                                         skip_group_check=True)
                        nc.tensor.matmul(outu[:, 512:520],
                                         lhsT=s01[:, ch, :],
                                         rhs=esttb[:, ch, :],
                                         start=fc, stop=lc,
                                         skip_group_check=True)
                else:
                    # fold w into the one-hot; denominator rides the const-1
                    # column (col 65) of the gathered row
                    nc.vector.tensor_tensor(
                        out=s01[:, 0:CH, :], in0=s01[:, 0:CH, :],
                        in1=esttb[:, 0:CH, 0:1].to_broadcast([P, CH, P]),
                        op=AT.mult)
                    for ch in range(CH):
                        fc = (g == 0 and ch == 0)
                        lc = (g == 1 and ch == CHb - 1)
                        nc.tensor.matmul(outu[:], lhsT=s01[:, ch, :],
                                         rhs=G[:, ch, 0:HID + 2],
                                         start=fc, stop=lc,
                                         skip_group_check=True)
            # ---- epilogue: normalize, bias, ELU ----
            if heads == 8:
                rec = wp.tile([P, 8], f32, tag="rec")
                nc.vector.reciprocal(rec[:], outu[:, 512:520])
                ho = op.tile([P, HC], f32, tag="ho")
                hov = ho[:].rearrange("p (c h) -> p c h", h=8)
                ouv = outu[:, 0:512].rearrange("p (c h) -> p c h", h=8)
                recb = (rec[:].to_broadcast([P, 8, 64])
                        .rearrange("p a b -> p b a"))
                nc.vector.tensor_tensor(out=hov, in0=ouv, in1=recb, op=AT.mult)
            else:
                rec = wp.tile([P, 1], f32, tag="rec")
                nc.vector.reciprocal(rec[:], outu[:, HID + 1:HID + 2])
                ho = op.tile([P, HC], f32, tag="ho")
                nc.vector.tensor_scalar(out=ho[:], in0=outu[:, 0:HID],
                                        scalar1=rec[:], scalar2=None,
                                        op0=AT.mult)
            nc.vector.tensor_tensor(out=ho[:], in0=ho[:], in1=brep[:],
                                    op=AT.add)
            el = op.tile([P, HC], f32, tag="el")
            nc.vector.tensor_scalar(out=el[:], in0=ho[:], scalar1=0.0,
                                    scalar2=None, op0=AT.min)
            nc.scalar.activation(el[:], el[:], AF.Exp)
            nc.vector.scalar_tensor_tensor(
                out=ho[:], in0=ho[:], scalar=0.0, in1=el[:],
                op0=AT.max, op1=AT.add)
            # single bf16 rounding of elu(...)-1 happens here, in one op
            hob = op.tile([P, HC], bf16, tag="hob")
            nc.scalar.activation(hob[:], ho[:], AF.Copy, bias=-1.0)
            # ---- PE transpose + fused next-layer matmul ----
            r = rows_of(t)
            if layer == 1:
                W2ap, ad2, hloc2 = nxt
                tsb = op.tile([P, 512], bf16, tag="tsb")
                for cb in range(4):
                    tp_ps = pp.tile([P, P], bf16, space="PSUM", tag="s01t")
                    nc.tensor.transpose(out=tp_ps[:],
                                        in_=hob[:, cb * P:(cb + 1) * P],
                                        identity=identb[:])
                    nc.vector.tensor_copy(tsb[:, cb * P:(cb + 1) * P],
                                          tp_ps[:])
                h2_ps = pa.tile([P, 66], f32, space="PSUM", tag="hnx")
                for cb in range(4):
                    nc.tensor.matmul(h2_ps[:], lhsT=tsb[:, cb * P:(cb + 1) * P],
                                     rhs=W2ap(cb), start=(cb == 0),
                                     stop=(cb == 3), skip_group_check=True)
                hc = wp.tile([P, R2], bf16, tag="hc")
                nc.vector.tensor_copy(hc[:, 0:65], h2_ps[:, 0:65])
                nc.vector.memset(hc[:, 65:66], 1.0)
                nc.vector.memset(hc[:, 66:R2], 0.0)
                nc.scalar.activation(ad2[:, t:t + 1], h2_ps[:, 65:66], AF.Copy)
                nc.sync.dma_start(out=hloc2[t * P:t * P + r, :], in_=hc[:r, :])
            elif layer == 2:
                W3ap, ad3, hloc3 = nxt
                tp_ps = pp.tile([P, P], bf16, space="PSUM", tag="s01t")
                nc.tensor.transpose(out=tp_ps[:HID, :], in_=hob[:],
                                    identity=identb[:])
                tsb = wp.tile([HID, P], bf16, tag="tsb64")
                nc.vector.tensor_copy(tsb[:], tp_ps[:HID, :])
                h3_ps = pa.tile([P, 66], f32, space="PSUM", tag="hnx")
                nc.tensor.matmul(h3_ps[:], lhsT=tsb[:], rhs=W3ap,
                                 start=True, stop=True)
                hc = wp.tile([P, R2], bf16, tag="hc")
                nc.vector.tensor_copy(hc[:, 0:65], h3_ps[:, 0:65])
                nc.vector.memset(hc[:, 65:66], 1.0)
                nc.vector.memset(hc[:, 66:R2], 0.0)
                nc.scalar.activation(ad3[:, t:t + 1], h3_ps[:, 65:66], AF.Copy)
                nc.sync.dma_start(out=hloc3[t * P:t * P + r, :], in_=hc[:r, :])
            else:
                Wcb, bcr, out_d = nxt
                tp_ps = pp.tile([P, P], bf16, space="PSUM", tag="s01t")
                nc.tensor.transpose(out=tp_ps[:HID, :], in_=hob[:],
                                    identity=identb[:])
                tsb = wp.tile([HID, P], bf16, tag="tsb64")
                nc.vector.tensor_copy(tsb[:], tp_ps[:HID, :])
                o_ps = pa.tile([P, D_OUT], f32, space="PSUM", tag="hnx")
                nc.tensor.matmul(o_ps[:], lhsT=tsb[:], rhs=Wcb[:],
                                 start=True, stop=True)
                ob = wp.tile([P, D_OUT], f32, tag="ob")
                nc.vector.tensor_tensor(out=ob[:], in0=o_ps[:], in1=bcr[:],
                                        op=AT.add)
                nc.sync.dma_start(out=out_d[t * P:t * P + r, :], in_=ob[:r, :])


PHASE_ORDER = ["m1", "ag1", "e1", "ag2", "e2", "ag3", "e3"]


def _build_program(chm, meta):
    stop = os.environ.get("GAT_STOP", "e3")
    lvl = PHASE_ORDER.index(stop) + 1
    nc = bacc.Bacc("TRN2", target_bir_lowering=False, debug=False,
                   enable_asserts=False, num_devices=NCORES,
                   num_swdge_queues=NQ)

    xT_in = nc.dram_tensor("xT", [P, NTP], bf16, kind="ExternalInput")
    idxA_in = nc.dram_tensor("idxA", [NT, P, chm * 8], i16, kind="ExternalInput")
    idxB_in = nc.dram_tensor("idxB", [NT, P, chm * 8], i16, kind="ExternalInput")
    dstA_in = nc.dram_tensor("dstA", [NT, P, chm], bf16, kind="ExternalInput")
    dstB_in = nc.dram_tensor("dstB", [NT, P, chm], bf16, kind="ExternalInput")
    dstTA_in = nc.dram_tensor("dstTA", [NT, chm * P], bf16, kind="ExternalInput")
    dstTB_in = nc.dram_tensor("dstTB", [NT, chm * P], bf16, kind="ExternalInput")
    W1Tp_in = nc.dram_tensor("W1Tp", [D_IN, 512], bf16, kind="ExternalInput")
    M1sd_in = nc.dram_tensor("M1sd", [D_IN, 16], bf16, kind="ExternalInput")
    W2a_in = nc.dram_tensor("W2a", [512, 66], bf16, kind="ExternalInput")
    W3a_in = nc.dram_tensor("W3a", [HID, 66], bf16, kind="ExternalInput")
    Wcb_in = nc.dram_tensor("Wcb", [HID, D_OUT], bf16, kind="ExternalInput")
    b1p_in = nc.dram_tensor("b1p", [P, 512], f32, kind="ExternalInput")
    b2r_in = nc.dram_tensor("b2r", [P, HID], f32, kind="ExternalInput")
    b3r_in = nc.dram_tensor("b3r", [P, HID], f32, kind="ExternalInput")
    bcr_in = nc.dram_tensor("bcr", [P, D_OUT], f32, kind="ExternalInput")

    out_d = nc.dram_tensor("out", [NTP, D_OUT], f32, kind="ExternalOutput")

    dbg = os.environ.get("GAT_DEBUG") == "1"
    hcat1_loc = nc.dram_tensor("hcat1_loc", [VP, R1], bf16, kind="Internal")
    hcat1_full = nc.dram_tensor("hcat1_full", [N, R1], bf16, kind="Internal",
                                addr_space="Shared")
    hcat2_loc = nc.dram_tensor("hcat2_loc", [VP, R2], bf16, kind="Internal")
    hcat2_full = nc.dram_tensor("hcat2_full", [N, R2], bf16, kind="Internal",
                                addr_space="Shared")
    hcat3_loc = nc.dram_tensor("hcat3_loc", [VP, R2], bf16, kind="Internal")
    hcat3_full = nc.dram_tensor("hcat3_full", [N, R2], bf16, kind="Internal",
                                addr_space="Shared")

    if dbg:
        dbg1 = nc.dram_tensor("dbg1", [VP, R1], bf16, kind="ExternalOutput")
        dbg2 = nc.dram_tensor("dbg2", [VP, R2], bf16, kind="ExternalOutput")
        dbg3 = nc.dram_tensor("dbg3", [VP, R2], bf16, kind="ExternalOutput")

    def rows_of(t):
        return P if t < NT - 1 else VP - (NT - 1) * P

    rg = [list(range(NCORES))]

    with tile.TileContext(nc) as tc:
        with tc.tile_pool(name="const", bufs=1) as cs:
            identb = cs.tile([P, P], bf16)
            make_identity(nc, identb[:])
            iota32 = cs.tile([P, P], f32)
            nc.gpsimd.iota(iota32[:], pattern=[[1, P]], base=0,
                           channel_multiplier=0,
                           allow_small_or_imprecise_dtypes=True)
            iotab = cs.tile([P, P], bf16)
            nc.vector.tensor_copy(iotab[:], iota32[:])
            iota_col = cs.tile([P, 1], f32)
            nc.gpsimd.iota(iota_col[:], pattern=[[0, 1]], base=0,
                           channel_multiplier=1,
                           allow_small_or_imprecise_dtypes=True)
            iota_rep = cs.tile([P, chm, P], bf16)
            for c in range(chm):
                nc.vector.tensor_copy(iota_rep[:, c, :], iotab[:])

            def c_load(name, shape, src, dtype=bf16):
                tl = cs.tile(shape, dtype, tag=name)
                nc.sync.dma_start(out=tl[:], in_=src)
                return tl

            W1Tp = c_load("W1Tp", [D_IN, 512], W1Tp_in[:])
            M1sd = c_load("M1sd", [D_IN, 16], M1sd_in[:])
            W2a = cs.tile([P, 4 * 66], bf16)
            for cb in range(4):
                nc.sync.dma_start(out=W2a[:, cb * 66:(cb + 1) * 66],
                                  in_=W2a_in[cb * P:(cb + 1) * P, :])
            W3a = c_load("W3a", [HID, 66], W3a_in[:])
            Wcb = c_load("Wcb", [HID, D_OUT], Wcb_in[:])
            b1p = c_load("b1p", [P, 512], b1p_in[:], dtype=f32)
            b2r = c_load("b2r", [P, HID], b2r_in[:], dtype=f32)
            b3r = c_load("b3r", [P, HID], b3r_in[:], dtype=f32)
            bcr = c_load("bcr", [P, D_OUT], bcr_in[:], dtype=f32)
            ad1 = cs.tile([P, NT * 8], bf16)
            ad2 = cs.tile([P, NT], bf16)
            ad3 = cs.tile([P, NT], bf16)

            # ---- M1: h1 = x @ W1.T (interleaved cols) + attn projections ----
            if lvl >= 1:
             with tc.tile_pool(name="m1", bufs=3) as mp, \
                 tc.tile_pool(name="m1x", bufs=1) as mxp, \
                 tc.tile_pool(name="m1p", bufs=2, space="PSUM") as mpp:
                xall = mxp.tile([P, NTP], bf16, tag="xall")
                nc.sync.dma_start(out=xall[:], in_=xT_in[:])
                for t in range(NT):
                    xt = xall[:, t * P:(t + 1) * P]
                    h_ps = mpp.tile([P, 512], f32, space="PSUM", tag="h")
                    nc.tensor.matmul(h_ps[:], lhsT=xt, rhs=W1Tp[:],
                                     start=True, stop=True)
                    aa_ps = mpp.tile([P, 16], f32, space="PSUM", tag="aa")
                    nc.tensor.matmul(aa_ps[:], lhsT=xt, rhs=M1sd[:],
                                     start=True, stop=True)
                    hc = mp.tile([P, R1], bf16, tag="hc")
                    nc.vector.tensor_copy(hc[:, 0:512], h_ps[:])
                    nc.scalar.activation(hc[:, 512:520], aa_ps[:, 0:8],
                                         AF.Copy)
                    nc.vector.memset(hc[:, 520:R1], 0.0)
                    nc.scalar.activation(ad1[:, t * 8:(t + 1) * 8],
                                         aa_ps[:, 8:16], AF.Copy)
                    r = rows_of(t)
                    nc.sync.dma_start(out=hcat1_loc[t * P:t * P + r, :],
                                      in_=hc[:r, :])
            if dbg:
                nc.sync.dma_start(out=dbg1[:], in_=hcat1_loc[:])
            if lvl >= 2:
             nc.gpsimd.collective_compute(
                "AllGather", AT.bypass, replica_groups=rg,
                ins=[hcat1_loc[:]], outs=[hcat1_full[:]])

            if lvl >= 3:
             _edge_phase(nc, tc, 1, chm, meta, (idxA_in, idxB_in),
                        (dstA_in, dstB_in), (dstTA_in, dstTB_in),
                        hcat1_full, R1, 8, ad1, iota_col, iota_rep, identb,
                        b1p, rows_of,
                        (lambda cb: W2a[:, cb * 66:(cb + 1) * 66], ad2,
                         hcat2_loc))
            if dbg and lvl >= 3:
                nc.sync.dma_start(out=dbg2[:], in_=hcat2_loc[:])
            if lvl >= 4:
             nc.gpsimd.collective_compute(
                "AllGather", AT.bypass, replica_groups=rg,
                ins=[hcat2_loc[:]], outs=[hcat2_full[:]])

            if lvl >= 5:
             _edge_phase(nc, tc, 2, chm, meta, (idxA_in, idxB_in),
                        (dstA_in, dstB_in), (dstTA_in, dstTB_in),
                        hcat2_full, R2, 1, ad2, iota_col, iota_rep, identb,
                        b2r, rows_of,
                        (W3a[:], ad3, hcat3_loc))
            if dbg and lvl >= 5:
                nc.sync.dma_start(out=dbg3[:], in_=hcat3_loc[:])
            if lvl >= 6:
             nc.gpsimd.collective_compute(
                "AllGather", AT.bypass, replica_groups=rg,
                ins=[hcat3_loc[:]], outs=[hcat3_full[:]])

            if lvl >= 7:
             _edge_phase(nc, tc, 3, chm, meta, (idxA_in, idxB_in),
                        (dstA_in, dstB_in), (dstTA_in, dstTB_in),
                        hcat3_full, R2, 1, ad3, iota_col, iota_rep, identb,
                        b3r, rows_of,
                        (Wcb, bcr, out_d))

    nc.compile()
    return nc


def prepare(**inputs):
    """Host preprocessing + program build; returns (nc, in_maps)."""
    x = np.asarray(inputs["x"], np.float32)
    edge_index = np.asarray(inputs["edge_index"])
    W1 = np.asarray(inputs["W1"], np.float32)
    a1_src = np.asarray(inputs["a1_src"], np.float32)
    a1_dst = np.asarray(inputs["a1_dst"], np.float32)
    b1 = np.asarray(inputs["b1"], np.float32)
    W2 = np.asarray(inputs["W2"], np.float32)
    a2_src = np.asarray(inputs["a2_src"], np.float32)
    a2_dst = np.asarray(inputs["a2_dst"], np.float32)
    b2 = np.asarray(inputs["b2"], np.float32)
    W3 = np.asarray(inputs["W3"], np.float32)
    a3_src = np.asarray(inputs["a3_src"], np.float32)
    a3_dst = np.asarray(inputs["a3_dst"], np.float32)
    b3 = np.asarray(inputs["b3"], np.float32)
    Wc = np.asarray(inputs["Wc"], np.float32)
    bc = np.asarray(inputs["bc"], np.float32)

    chm, meta, idx_arrs, dst_arrs, dstT_arrs = _prep_edges(edge_index)

    # interleave permutation: new col c*8+h <- old col h*64+c
    jn = np.arange(512)
    old_idx = (jn % 8) * 64 + jn // 8

    W1T = W1.T                                     # [128, 512]
    W1h = W1.reshape(HEADS, HID, D_IN)
    M1s = np.einsum("hci,hc->ih", W1h, a1_src)     # [128, 8]
    M1d = np.einsum("hci,hc->ih", W1h, a1_dst)
    W2T = W2.T                                     # [512, 64]
    M2s = W2.T @ a2_src[0]                         # [512]
    M2d = W2.T @ a2_dst[0]
    W3T = W3.T
    M3s = W3.T @ a3_src[0]
    M3d = W3.T @ a3_dst[0]

    common = {
        "W1Tp": np.ascontiguousarray(W1T[:, old_idx]).astype(BF),
        "M1sd": np.concatenate([M1s, M1d], 1).astype(BF),
        "W2a": np.concatenate(
            [W2T, M2s[:, None], M2d[:, None]], 1)[old_idx, :].astype(BF),
        "W3a": np.concatenate(
            [W3T, M3s[:, None], M3d[:, None]], 1).astype(BF),
        "Wcb": np.ascontiguousarray(Wc.T).astype(BF),
        "b1p": np.tile(b1[old_idx], (P, 1)).astype(np.float32),
        "b2r": np.tile(b2, (P, 1)).astype(np.float32),
        "b3r": np.tile(b3, (P, 1)).astype(np.float32),
        "bcr": np.tile(bc, (P, 1)).astype(np.float32),
    }

    in_maps = []
    for k in range(NCORES):
        xk = x[k * VP:(k + 1) * VP]
        xT = np.zeros((P, NTP), np.float32)
        xT[:, :VP] = xk.T
        m = dict(common)
        m["xT"] = xT.astype(BF)
        m["idxA"] = idx_arrs[0][k]
        m["idxB"] = idx_arrs[1][k]
        m["dstA"] = dst_arrs[0][k]
        m["dstB"] = dst_arrs[1][k]
        m["dstTA"] = dstT_arrs[0][k]
        m["dstTB"] = dstT_arrs[1][k]
        in_maps.append(m)

    nc = _build_program(chm, meta)
    return nc, in_maps


def kernel(**inputs):
    nc, in_maps = prepare(**inputs)
    r = run_bass_kernel_spmd(nc, in_maps, core_ids=list(range(NCORES)))
    out = np.concatenate([r.results[k]["out"][:VP] for k in range(NCORES)], 0)
    return out.astype(np.float32)


# revision 9
# speedup vs baseline: 1.3719x; 1.0207x over previous
"""3-layer GAT + linear head on 8 TRN2 NeuronCores (Bass/Tile), bf16 edition.

Sharding (follows the problem hint):
  - Nodes split into 8 contiguous blocks of 6250; core k owns block k and
    every edge whose destination lies in its block.
  - Per layer: each core computes H = X @ W.T (+ attention projections) for
    its own nodes, AllGathers the rows into a replicated bf16 node table,
    then processes its edges per 128-destination tile:
      * dma_gather of bf16 source rows (round-robin over 4 SWDGE queues),
      * one-hot scatter matrices s01 / s01T built with ONE batched vector op
        each per tile-group (is_equal against an iota-repeat tile resp. a
        DMA-broadcast per-edge dst-slot row),
      * a_d[dst] broadcast to edges via per-chunk matmuls (lhsT = s01T
        slice) accumulated into a single PSUM strip; batched leakyrelu+exp,
      * segment softmax + weighted sum via one-hot matmuls on TensorE; for
        layers 2/3 the edge weight w is folded into the one-hot (sW) and the
        softmax denominator rides a constant-1 column of the gathered row,
      * epilogue: normalize, bias, ELU, PE-transpose, and the NEXT layer's
        X @ W.T fused in (h1T/h2T/h3T never round-trip DRAM).
  - Gather descriptor generation (the gpsimd bottleneck) is trimmed with
    per-tile chunk counts (max over cores so the program stays SPMD), -1
    tail indices (skipped by the HW), and exact num_idxs_reg.
  - Layer-1 h uses an interleaved (channel, head) column order so the
    per-edge/-dst broadcasts are unit-stride on the vector engine; weight
    matrices are permuted host-side to compensate.

Self-contained; hardcodes shapes for N=50000, E=800000, D_IN=128, HID=64,
HEADS=8, D_OUT=10.
"""
import os
import numpy as np
import ml_dtypes

import concourse.bass as bass
import concourse.mybir as mybir
import concourse.tile as tile
from concourse import bacc
from concourse.bass_utils import run_bass_kernel_spmd
from concourse.masks import make_identity

N = 50000
E = 800000
NCORES = 8
VP = N // NCORES          # 6250 nodes per core
P = 128
NT = (VP + P - 1) // P    # 49 dst tiles per core (last has 106 rows)
NTP = NT * P              # 6272
HALF = N // 2             # 25000
D_IN = 128
HID = 64
HEADS = 8
D_OUT = 10
R1 = 640                  # layer-1 row: h(512 interleaved) | a_s(8) | pad
R2 = 128                  # layer-2/3 row: h(64) | a_s(1) | one(1) | pad
NQ = 4                    # swdge queues for gather descriptor generation

f32 = mybir.dt.float32
bf16 = mybir.dt.bfloat16
i16 = mybir.dt.int16
AT = mybir.AluOpType
AF = mybir.ActivationFunctionType

BF = ml_dtypes.bfloat16

SP23 = os.environ.get("GAT_SP", "0") == "1"   # single_packet for 256B rows
                                              # (hangs the NRT as of now)
USE4D = os.environ.get("GAT_4D", "1") == "1"  # batched 4D G*w multiply in e1


def _prep_edges(edge_index):
    src = np.concatenate([np.asarray(edge_index[0]), np.arange(N)]).astype(np.int64)
    dst = np.concatenate([np.asarray(edge_index[1]), np.arange(N)]).astype(np.int64)

    # per_core[k][t][g] = (local_src_idx, dst_slot) arrays
    per_core = []
    cnts = np.zeros((NCORES, NT, 2), np.int64)
    for k in range(NCORES):
        m = (dst >= k * VP) & (dst < (k + 1) * VP)
        s_k = src[m]
        dloc = dst[m] - k * VP
        t_k = dloc // P
        w_k = dloc % P
        tiles = []
        for t in range(NT):
            sel = t_k == t
            ss, ww = s_k[sel], w_k[sel]
            groups = []
            for g in range(2):
                gm = (ss < HALF) if g == 0 else (ss >= HALF)
                li = (ss[gm] - g * HALF).astype(np.int64)
                groups.append((li, ww[gm].astype(np.int64)))
                cnts[k, t, g] = len(li)
            tiles.append(groups)
        per_core.append(tiles)

    # cnt_max[t][g] identical across cores so the compiled program is SPMD
    cnt_max = np.maximum(cnts.max(axis=0), 1)        # [NT, 2]
    ch_t = (cnt_max + P - 1) // P                    # [NT, 2]
    chm = int(ch_t.max())

    # meta[t][g] = (CH_t, cnt_max) ints
    meta = [[(int(ch_t[t, g]), int(cnt_max[t, g])) for g in range(2)]
            for t in range(NT)]

    idx_arrs = [[], []]
    dst_arrs = [[], []]
    dstT_arrs = [[], []]
    for g in range(2):
        for k in range(NCORES):
            A = np.full((NT, P, chm * 8), -1, np.int16)
            D = np.full((NT, P, chm), -1.0, np.float32)
            DT = np.full((NT, chm * P), -1.0, np.float32)
            for t in range(NT):
                li, ww = per_core[k][t][g]
                n = len(li)
                ch, cm = meta[t][g]
                nid = ch * P
                # idx stream: valid edges, then zero-pads (valid) to cnt_max,
                # then -1 (skipped by HW) to CH_t*128
                iv = np.full(nid, -1, np.int16)
                iv[:n] = li.astype(np.int16)
                iv[n:cm] = 0
                ii = np.arange(nid)
                wrap = np.zeros((16, nid // 16), np.int16)
                wrap[ii % 16, ii // 16] = iv
                A[t, :, 0:nid // 16] = np.tile(wrap, (8, 1))
                ie = np.arange(n)
                D[t, ie % P, ie // P] = ww
                DT[t, 0:n] = ww
            idx_arrs[g].append(A)
            dst_arrs[g].append(D.astype(BF))
            dstT_arrs[g].append(DT.astype(BF))

    return chm, meta, idx_arrs, dst_arrs, dstT_arrs


def _edge_phase(nc, tc, layer, chm, meta, idx_ins, dst_ins, dstT_ins, hfull,
                Rrow, heads, ad_sb, iota_colb, iota_rep, identb, brep, rows_of,
                nxt):
    """Edge aggregation for one GAT layer + fused next-layer matmul.

    nxt: (W_next_ap_fn, ad_next, hloc_next) for layers 1/2 where
         W_next_ap_fn(cb) yields the [128, 66] rhs block; for layer 3:
         (Wcb, bcr, out_d).
    """
    HC = 512 if layer == 1 else HID
    sp = (Rrow * 2 == 256) and SP23
    with tc.tile_pool(name=f"e{layer}", bufs=8) as ep, \
         tc.tile_pool(name=f"e{layer}o", bufs=2) as op, \
         tc.tile_pool(name=f"e{layer}w", bufs=8) as wp, \
         tc.tile_pool(name=f"e{layer}dt", bufs=4) as dp, \
         tc.tile_pool(name=f"e{layer}s", bufs=4) as s01p, \
         tc.tile_pool(name=f"e{layer}p1", bufs=2, space="PSUM") as pp, \
         tc.tile_pool(name=f"e{layer}p2", bufs=2, space="PSUM") as pa, \
         tc.tile_pool(name=f"e{layer}p3", bufs=1 if heads == 8 else 2,
                      space="PSUM") as po:
        PF = 3  # idx/dst prefetch distance (tiles)
        pend = {}

        def load_tile(tt):
            for g in (0, 1):
                CH, _cm = meta[tt][g]
                idxt = wp.tile([P, chm * 8], i16, tag="idx")
                nc.sync.dma_start(out=idxt[:, 0:CH * 8],
                                  in_=idx_ins[g][tt, :, 0:CH * 8])
                dstt = wp.tile([P, chm], bf16, tag="dst")
                nc.sync.dma_start(out=dstt[:, 0:CH], in_=dst_ins[g][tt, :, 0:CH])
                dstTb = dp.tile([P, chm * P], bf16, tag="dstT")
                nc.sync.dma_start(
                    out=dstTb[:, 0:CH * P],
                    in_=dstT_ins[g][tt:tt + 1, 0:CH * P]
                    .to_broadcast([P, CH * P]))
                pend[(tt, g)] = (idxt, dstt, dstTb)

        for tt in range(min(PF, NT)):
            load_tile(tt)
        for t in range(NT):
            if t + PF < NT:
                load_tile(t + PF)
            if heads == 8:
                # cols 0:512 numerator, 512:520 softmax denominator (ssum)
                outu = po.tile([P, HC + 8], f32, space="PSUM", tag="outu")
            else:
                # cols 0:64 numerator, 64 = sum(w*a_s) (unused), 65 = sum(w)
                outu = po.tile([P, HID + 2], f32, space="PSUM", tag="outu")
            adT = ad_sb[:, t * heads:(t + 1) * heads]
            CHb = meta[t][1][0]
            for g in range(2):
                CH, cm = meta[t][g]
                NIDX = CH * P
                idxt, dstt, dstTb = pend.pop((t, g))
                G = ep.tile([P, chm, Rrow], bf16, tag="G")
                if 2 * t + g < 8:
                    # first pass through the 8 G buffers: zero them so
                    # skipped (-1) rows never expose NaN bit patterns
                    nc.vector.memset(G[:], 0.0)
                half = hfull[g * HALF:(g + 1) * HALF, :]
                nc.gpsimd.dma_gather(G[:, 0:CH, :], half, idxt[:, 0:CH * 8],
                                     NIDX, cm, Rrow, single_packet=sp,
                                     queue_num=(2 * t + g) % NQ)
                # --- batched one-hot builds ---
                s01 = s01p.tile([P, chm, P], bf16, tag="s01")
                nc.vector.tensor_tensor(
                    out=s01[:, 0:CH, :], in0=iota_rep[:, 0:CH, :],
                    in1=dstt[:, 0:CH, None].to_broadcast([P, CH, P]),
                    op=AT.is_equal)
                s01T = s01p.tile([P, chm * P], bf16, tag="s01T")
                nc.vector.tensor_tensor(
                    out=s01T[:, 0:CH * P], in0=dstTb[:, 0:CH * P],
                    in1=iota_colb[:].to_broadcast([P, CH * P]),
                    op=AT.is_equal)
                # --- a_d[dst] -> edges via matmuls into one PSUM strip ---
                estt_ps = pa.tile([P, chm * heads], f32, space="PSUM",
                                  tag="estt")
                for ch in range(CH):
                    nc.tensor.matmul(estt_ps[:, ch * heads:(ch + 1) * heads],
                                     lhsT=s01T[:, ch * P:(ch + 1) * P],
                                     rhs=adT, start=True, stop=True,
                                     skip_group_check=True)
                # --- e = leakyrelu(a_s + a_d); w = exp(e) (batched) ---
                estt = wp.tile([P, chm, heads], f32, tag="estt_sb")
                nc.vector.tensor_tensor(
                    out=estt[:, 0:CH, :],
                    in0=G[:, 0:CH, HC:HC + heads],
                    in1=estt_ps[:].rearrange("p (c h) -> p c h",
                                             h=heads)[:, 0:CH, :],
                    op=AT.add)
                ef = estt[:, 0:CH, :]
                nc.vector.scalar_tensor_tensor(
                    out=ef, in0=ef, scalar=0.2, in1=ef,
                    op0=AT.mult, op1=AT.max)
                esttb = wp.tile([P, chm, heads], bf16, tag="esttb")
                nc.scalar.activation(esttb[:, 0:CH, :], ef, AF.Exp)
                # --- weighted scatter-sum ---
                if heads == 8:
                    if USE4D:
                        gv = G[:, 0:CH, 0:512].rearrange(
                            "p c (a h) -> p c a h", h=8)
                        wv = (esttb[:, 0:CH, None, :]
                              .to_broadcast([P, CH, 64, 8]))
                        nc.vector.tensor_tensor(out=gv, in0=gv, in1=wv,
                                                op=AT.mult)
                    else:
                        for ch in range(CH):
                            gv = G[:, ch, 0:512].rearrange(
                                "p (c h) -> p c h", h=8)
                            wv = (esttb[:, ch, :].to_broadcast([P, 8, 64])
                                  .rearrange("p a b -> p b a"))
                            nc.vector.tensor_tensor(out=gv, in0=gv, in1=wv,
                                                    op=AT.mult)
                    for ch in range(CH):
                        fc = (g == 0 and ch == 0)
                        lc = (g == 1 and ch == CHb - 1)
                        nc.tensor.matmul(outu[:, 0:512], lhsT=s01[:, ch, :],
                                         rhs=G[:, ch, 0:512],
                                         start=fc, stop=lc,
                                         skip_group_check=True)
                        nc.tensor.matmul(outu[:, 512:520],
                                         lhsT=s01[:, ch, :],
                                         rhs=esttb[:, ch, :],
                                         start=fc, stop=lc,
                                         skip_group_check=True)
                else:
                    # fold w into the one-hot; denominator rides the const-1
                    # column (col 65) of the gathered row
                    nc.vector.tensor_tensor(
                        out=s01[:, 0:CH, :], in0=s01[:, 0:CH, :],
                        in1=esttb[:, 0:CH, 0:1].to_broadcast([P, CH, P]),
                        op=AT.mult)
                    for ch in range(CH):
                        fc = (g == 0 and ch == 0)
                        lc = (g == 1 and ch == CHb - 1)
                        nc.tensor.matmul(outu[:], lhsT=s01[:, ch, :],
                                         rhs=G[:, ch, 0:HID + 2],
                                         start=fc, stop=lc,
                                         skip_group_check=True)
            # ---- epilogue: normalize, bias, ELU ----
            if heads == 8:
                rec = wp.tile([P, 8], f32, tag="rec")
                nc.vector.reciprocal(rec[:], outu[:, 512:520])
                ho = op.tile([P, HC], f32, tag="ho")
                hov = ho[:].rearrange("p (c h) -> p c h", h=8)
                ouv = outu[:, 0:512].rearrange("p (c h) -> p c h", h=8)
                recb = (rec[:].to_broadcast([P, 8, 64])
                        .rearrange("p a b -> p b a"))
                nc.vector.tensor_tensor(out=hov, in0=ouv, in1=recb, op=AT.mult)
            else:
                rec = wp.tile([P, 1], f32, tag="rec")
                nc.vector.reciprocal(rec[:], outu[:, HID + 1:HID + 2])
                ho = op.tile([P, HC], f32, tag="ho")
                nc.vector.tensor_scalar(out=ho[:], in0=outu[:, 0:HID],
                                        scalar1=rec[:], scalar2=None,
                                        op0=AT.mult)
            nc.vector.tensor_tensor(out=ho[:], in0=ho[:], in1=brep[:],
                                    op=AT.add)
            el = op.tile([P, HC], f32, tag="el")
            nc.vector.tensor_scalar(out=el[:], in0=ho[:], scalar1=0.0,
                                    scalar2=None, op0=AT.min)
            nc.scalar.activation(el[:], el[:], AF.Exp)
            nc.vector.scalar_tensor_tensor(
                out=ho[:], in0=ho[:], scalar=0.0, in1=el[:],
                op0=AT.max, op1=AT.add)
            # single bf16 rounding of elu(...)-1 happens here, in one op
            hob = op.tile([P, HC], bf16, tag="hob")
            nc.scalar.activation(hob[:], ho[:], AF.Copy, bias=-1.0)
            # ---- PE transpose + fused next-layer matmul ----
            r = rows_of(t)
            if layer == 1:
                W2ap, ad2, hloc2 = nxt
                tsb = op.tile([P, 512], bf16, tag="tsb")
                for cb in range(4):
                    tp_ps = pp.tile([P, P], bf16, space="PSUM", tag="s01t")
                    nc.tensor.transpose(out=tp_ps[:],
                                        in_=hob[:, cb * P:(cb + 1) * P],
                                        identity=identb[:])
                    nc.vector.tensor_copy(tsb[:, cb * P:(cb + 1) * P],
                                          tp_ps[:])
                h2_ps = pa.tile([P, 66], f32, space="PSUM", tag="hnx")
                for cb in range(4):
                    nc.tensor.matmul(h2_ps[:], lhsT=tsb[:, cb * P:(cb + 1) * P],
                                     rhs=W2ap(cb), start=(cb == 0),
                                     stop=(cb == 3), skip_group_check=True)
                hc = wp.tile([P, R2], bf16, tag="hc")
                nc.vector.tensor_copy(hc[:, 0:65], h2_ps[:, 0:65])
                nc.vector.memset(hc[:, 65:66], 1.0)
                nc.vector.memset(hc[:, 66:R2], 0.0)
                nc.scalar.activation(ad2[:, t:t + 1], h2_ps[:, 65:66], AF.Copy)
                nc.sync.dma_start(out=hloc2[t * P:t * P + r, :], in_=hc[:r, :])
            elif layer == 2:
                W3ap, ad3, hloc3 = nxt
                tp_ps = pp.tile([P, P], bf16, space="PSUM", tag="s01t")
                nc.tensor.transpose(out=tp_ps[:HID, :], in_=hob[:],
                                    identity=identb[:])
                tsb = wp.tile([HID, P], bf16, tag="tsb64")
                nc.vector.tensor_copy(tsb[:], tp_ps[:HID, :])
                h3_ps = pa.tile([P, 66], f32, space="PSUM", tag="hnx")
                nc.tensor.matmul(h3_ps[:], lhsT=tsb[:], rhs=W3ap,
                                 start=True, stop=True)
                hc = wp.tile([P, R2], bf16, tag="hc")
                nc.vector.tensor_copy(hc[:, 0:65], h3_ps[:, 0:65])
                nc.vector.memset(hc[:, 65:66], 1.0)
                nc.vector.memset(hc[:, 66:R2], 0.0)
                nc.scalar.activation(ad3[:, t:t + 1], h3_ps[:, 65:66], AF.Copy)
                nc.sync.dma_start(out=hloc3[t * P:t * P + r, :], in_=hc[:r, :])
            else:
                Wcb, bcr, out_d = nxt
                tp_ps = pp.tile([P, P], bf16, space="PSUM", tag="s01t")
                nc.tensor.transpose(out=tp_ps[:HID, :], in_=hob[:],
                                    identity=identb[:])
                tsb = wp.tile([HID, P], bf16, tag="tsb64")
                nc.vector.tensor_copy(tsb[:], tp_ps[:HID, :])
                o_ps = pa.tile([P, D_OUT], f32, space="PSUM", tag="hnx")
                nc.tensor.matmul(o_ps[:], lhsT=tsb[:], rhs=Wcb[:],
                                 start=True, stop=True)
                ob = wp.tile([P, D_OUT], f32, tag="ob")
                nc.vector.tensor_tensor(out=ob[:], in0=o_ps[:], in1=bcr[:],
                                        op=AT.add)
                nc.sync.dma_start(out=out_d[t * P:t * P + r, :], in_=ob[:r, :])


PHASE_ORDER = ["m1", "ag1", "e1", "ag2", "e2", "ag3", "e3"]


def _build_program(chm, meta):
    stop = os.environ.get("GAT_STOP", "e3")
    lvl = PHASE_ORDER.index(stop) + 1
    nc = bacc.Bacc("TRN2", target_bir_lowering=False, debug=False,
                   enable_asserts=False, num_devices=NCORES,
                   num_swdge_queues=NQ)

    xT_in = nc.dram_tensor("xT", [P, NTP], bf16, kind="ExternalInput")
    idxA_in = nc.dram_tensor("idxA", [NT, P, chm * 8], i16, kind="ExternalInput")
    idxB_in = nc.dram_tensor("idxB", [NT, P, chm * 8], i16, kind="ExternalInput")
    dstA_in = nc.dram_tensor("dstA", [NT, P, chm], bf16, kind="ExternalInput")
    dstB_in = nc.dram_tensor("dstB", [NT, P, chm], bf16, kind="ExternalInput")
    dstTA_in = nc.dram_tensor("dstTA", [NT, chm * P], bf16, kind="ExternalInput")
    dstTB_in = nc.dram_tensor("dstTB", [NT, chm * P], bf16, kind="ExternalInput")
    W1Tp_in = nc.dram_tensor("W1Tp", [D_IN, 512], bf16, kind="ExternalInput")
    M1sd_in = nc.dram_tensor("M1sd", [D_IN, 16], bf16, kind="ExternalInput")
    W2a_in = nc.dram_tensor("W2a", [512, 66], bf16, kind="ExternalInput")
    W3a_in = nc.dram_tensor("W3a", [HID, 66], bf16, kind="ExternalInput")
    Wcb_in = nc.dram_tensor("Wcb", [HID, D_OUT], bf16, kind="ExternalInput")
    b1p_in = nc.dram_tensor("b1p", [P, 512], f32, kind="ExternalInput")
    b2r_in = nc.dram_tensor("b2r", [P, HID], f32, kind="ExternalInput")
    b3r_in = nc.dram_tensor("b3r", [P, HID], f32, kind="ExternalInput")
    bcr_in = nc.dram_tensor("bcr", [P, D_OUT], f32, kind="ExternalInput")

    out_d = nc.dram_tensor("out", [NTP, D_OUT], f32, kind="ExternalOutput")

    dbg = os.environ.get("GAT_DEBUG") == "1"
    hcat1_loc = nc.dram_tensor("hcat1_loc", [VP, R1], bf16, kind="Internal")
    hcat1_full = nc.dram_tensor("hcat1_full", [N, R1], bf16, kind="Internal",
                                addr_space="Shared")
    hcat2_loc = nc.dram_tensor("hcat2_loc", [VP, R2], bf16, kind="Internal")
    hcat2_full = nc.dram_tensor("hcat2_full", [N, R2], bf16, kind="Internal",
                                addr_space="Shared")
    hcat3_loc = nc.dram_tensor("hcat3_loc", [VP, R2], bf16, kind="Internal")
    hcat3_full = nc.dram_tensor("hcat3_full", [N, R2], bf16, kind="Internal",
                                addr_space="Shared")

    if dbg:
        dbg1 = nc.dram_tensor("dbg1", [VP, R1], bf16, kind="ExternalOutput")
        dbg2 = nc.dram_tensor("dbg2", [VP, R2], bf16, kind="ExternalOutput")
        dbg3 = nc.dram_tensor("dbg3", [VP, R2], bf16, kind="ExternalOutput")

    def rows_of(t):
        return P if t < NT - 1 else VP - (NT - 1) * P

    rg = [list(range(NCORES))]

    with tile.TileContext(nc) as tc:
        with tc.tile_pool(name="const", bufs=1) as cs:
            identb = cs.tile([P, P], bf16)
            make_identity(nc, identb[:])
            iota32 = cs.tile([P, P], f32)
            nc.gpsimd.iota(iota32[:], pattern=[[1, P]], base=0,
                           channel_multiplier=0,
                           allow_small_or_imprecise_dtypes=True)
            iotab = cs.tile([P, P], bf16)
            nc.vector.tensor_copy(iotab[:], iota32[:])
            iota_col = cs.tile([P, 1], f32)
            nc.gpsimd.iota(iota_col[:], pattern=[[0, 1]], base=0,
                           channel_multiplier=1,
                           allow_small_or_imprecise_dtypes=True)
            iota_colb = cs.tile([P, 1], bf16)
            nc.vector.tensor_copy(iota_colb[:], iota_col[:])
            iota_rep = cs.tile([P, chm, P], bf16)
            for c in range(chm):
                nc.vector.tensor_copy(iota_rep[:, c, :], iotab[:])

            def c_load(name, shape, src, dtype=bf16):
                tl = cs.tile(shape, dtype, tag=name)
                nc.sync.dma_start(out=tl[:], in_=src)
                return tl

            W1Tp = c_load("W1Tp", [D_IN, 512], W1Tp_in[:])
            M1sd = c_load("M1sd", [D_IN, 16], M1sd_in[:])
            W2a = cs.tile([P, 4 * 66], bf16)
            for cb in range(4):
                nc.sync.dma_start(out=W2a[:, cb * 66:(cb + 1) * 66],
                                  in_=W2a_in[cb * P:(cb + 1) * P, :])
            W3a = c_load("W3a", [HID, 66], W3a_in[:])
            Wcb = c_load("Wcb", [HID, D_OUT], Wcb_in[:])
            b1p = c_load("b1p", [P, 512], b1p_in[:], dtype=f32)
            b2r = c_load("b2r", [P, HID], b2r_in[:], dtype=f32)
            b3r = c_load("b3r", [P, HID], b3r_in[:], dtype=f32)
            bcr = c_load("bcr", [P, D_OUT], bcr_in[:], dtype=f32)
            ad1 = cs.tile([P, NT * 8], bf16)
            ad2 = cs.tile([P, NT], bf16)
            ad3 = cs.tile([P, NT], bf16)

            # ---- M1: h1 = x @ W1.T (interleaved cols) + attn projections ----
            if lvl >= 1:
             with tc.tile_pool(name="m1", bufs=3) as mp, \
                 tc.tile_pool(name="m1x", bufs=1) as mxp, \
                 tc.tile_pool(name="m1p", bufs=2, space="PSUM") as mpp:
                xall = mxp.tile([P, NTP], bf16, tag="xall")
                nc.sync.dma_start(out=xall[:], in_=xT_in[:])
                for t in range(NT):
                    xt = xall[:, t * P:(t + 1) * P]
                    h_ps = mpp.tile([P, 512], f32, space="PSUM", tag="h")
                    nc.tensor.matmul(h_ps[:], lhsT=xt, rhs=W1Tp[:],
                                     start=True, stop=True)
                    aa_ps = mpp.tile([P, 16], f32, space="PSUM", tag="aa")
                    nc.tensor.matmul(aa_ps[:], lhsT=xt, rhs=M1sd[:],
                                     start=True, stop=True)
                    hc = mp.tile([P, R1], bf16, tag="hc")
                    nc.vector.tensor_copy(hc[:, 0:512], h_ps[:])
                    nc.scalar.activation(hc[:, 512:520], aa_ps[:, 0:8],
                                         AF.Copy)
                    nc.vector.memset(hc[:, 520:R1], 0.0)
                    nc.scalar.activation(ad1[:, t * 8:(t + 1) * 8],
                                         aa_ps[:, 8:16], AF.Copy)
                    r = rows_of(t)
                    nc.sync.dma_start(out=hcat1_loc[t * P:t * P + r, :],
                                      in_=hc[:r, :])
            if dbg:
                nc.sync.dma_start(out=dbg1[:], in_=hcat1_loc[:])
            if lvl >= 2:
             nc.gpsimd.collective_compute(
                "AllGather", AT.bypass, replica_groups=rg,
                ins=[hcat1_loc[:]], outs=[hcat1_full[:]])

            if lvl >= 3:
             _edge_phase(nc, tc, 1, chm, meta, (idxA_in, idxB_in),
                        (dstA_in, dstB_in), (dstTA_in, dstTB_in),
                        hcat1_full, R1, 8, ad1, iota_colb, iota_rep, identb,
                        b1p, rows_of,
                        (lambda cb: W2a[:, cb * 66:(cb + 1) * 66], ad2,
                         hcat2_loc))
            if dbg and lvl >= 3:
                nc.sync.dma_start(out=dbg2[:], in_=hcat2_loc[:])
            if lvl >= 4:
             nc.gpsimd.collective_compute(
                "AllGather", AT.bypass, replica_groups=rg,
                ins=[hcat2_loc[:]], outs=[hcat2_full[:]])

            if lvl >= 5:
             _edge_phase(nc, tc, 2, chm, meta, (idxA_in, idxB_in),
                        (dstA_in, dstB_in), (dstTA_in, dstTB_in),
                        hcat2_full, R2, 1, ad2, iota_colb, iota_rep, identb,
                        b2r, rows_of,
                        (W3a[:], ad3, hcat3_loc))
            if dbg and lvl >= 5:
                nc.sync.dma_start(out=dbg3[:], in_=hcat3_loc[:])
            if lvl >= 6:
             nc.gpsimd.collective_compute(
                "AllGather", AT.bypass, replica_groups=rg,
                ins=[hcat3_loc[:]], outs=[hcat3_full[:]])

            if lvl >= 7:
             _edge_phase(nc, tc, 3, chm, meta, (idxA_in, idxB_in),
                        (dstA_in, dstB_in), (dstTA_in, dstTB_in),
                        hcat3_full, R2, 1, ad3, iota_colb, iota_rep, identb,
                        b3r, rows_of,
                        (Wcb, bcr, out_d))

    nc.compile()
    return nc


def prepare(**inputs):
    """Host preprocessing + program build; returns (nc, in_maps)."""
    x = np.asarray(inputs["x"], np.float32)
    edge_index = np.asarray(inputs["edge_index"])
    W1 = np.asarray(inputs["W1"], np.float32)
    a1_src = np.asarray(inputs["a1_src"], np.float32)
    a1_dst = np.asarray(inputs["a1_dst"], np.float32)
    b1 = np.asarray(inputs["b1"], np.float32)
    W2 = np.asarray(inputs["W2"], np.float32)
    a2_src = np.asarray(inputs["a2_src"], np.float32)
    a2_dst = np.asarray(inputs["a2_dst"], np.float32)
    b2 = np.asarray(inputs["b2"], np.float32)
    W3 = np.asarray(inputs["W3"], np.float32)
    a3_src = np.asarray(inputs["a3_src"], np.float32)
    a3_dst = np.asarray(inputs["a3_dst"], np.float32)
    b3 = np.asarray(inputs["b3"], np.float32)
    Wc = np.asarray(inputs["Wc"], np.float32)
    bc = np.asarray(inputs["bc"], np.float32)

    chm, meta, idx_arrs, dst_arrs, dstT_arrs = _prep_edges(edge_index)

    # interleave permutation: new col c*8+h <- old col h*64+c
    jn = np.arange(512)
    old_idx = (jn % 8) * 64 + jn // 8

    W1T = W1.T                                     # [128, 512]
    W1h = W1.reshape(HEADS, HID, D_IN)
    M1s = np.einsum("hci,hc->ih", W1h, a1_src)     # [128, 8]
    M1d = np.einsum("hci,hc->ih", W1h, a1_dst)
    W2T = W2.T                                     # [512, 64]
    M2s = W2.T @ a2_src[0]                         # [512]
    M2d = W2.T @ a2_dst[0]
    W3T = W3.T
    M3s = W3.T @ a3_src[0]
    M3d = W3.T @ a3_dst[0]

    common = {
        "W1Tp": np.ascontiguousarray(W1T[:, old_idx]).astype(BF),
        "M1sd": np.concatenate([M1s, M1d], 1).astype(BF),
        "W2a": np.concatenate(
            [W2T, M2s[:, None], M2d[:, None]], 1)[old_idx, :].astype(BF),
        "W3a": np.concatenate(
            [W3T, M3s[:, None], M3d[:, None]], 1).astype(BF),
        "Wcb": np.ascontiguousarray(Wc.T).astype(BF),
        "b1p": np.tile(b1[old_idx], (P, 1)).astype(np.float32),
        "b2r": np.tile(b2, (P, 1)).astype(np.float32),
        "b3r": np.tile(b3, (P, 1)).astype(np.float32),
        "bcr": np.tile(bc, (P, 1)).astype(np.float32),
    }

    in_maps = []
    for k in range(NCORES):
        xk = x[k * VP:(k + 1) * VP]
        xT = np.zeros((P, NTP), np.float32)
        xT[:, :VP] = xk.T
        m = dict(common)
        m["xT"] = xT.astype(BF)
        m["idxA"] = idx_arrs[0][k]
        m["idxB"] = idx_arrs[1][k]
        m["dstA"] = dst_arrs[0][k]
        m["dstB"] = dst_arrs[1][k]
        m["dstTA"] = dstT_arrs[0][k]
        m["dstTB"] = dstT_arrs[1][k]
        in_maps.append(m)

    nc = _build_program(chm, meta)
    return nc, in_maps


def kernel(**inputs):
    nc, in_maps = prepare(**inputs)
    r = run_bass_kernel_spmd(nc, in_maps, core_ids=list(range(NCORES)))
    out = np.concatenate([r.results[k]["out"][:VP] for k in range(NCORES)], 0)
    return out.astype(np.float32)
